# revision 1
# baseline (speedup 1.0000x reference)
"""Trainium2 Bass kernel for the KernelAttention module.

Sharding: the 4096 query positions (H*W) are split into 8 contiguous
blocks of 512, one per NeuronCore. The softmax mixes only across
(camera, g) at a FIXED query position, so this split needs no
collectives: every core computes its 512 output rows end-to-end.

Device-side layout strategy (per core):
  - activations live as [128 rows, 256 feat] tiles (rows on partitions)
  - LayerNorm stats via bn_stats/bn_aggr (free-dim reduction, native)
  - normalized tiles are transposed on the TensorEngine (2x 128x128)
    to produce the lhsT operand for B1-orientation matmuls:
        psum[rows, dout] += xT[k-tile].T @ W[k-tile]
    with float32r (full-rate fp32 matmul mode, moving dim >= 256)
  - LN gain and the attention 1/sqrt(dh) scale are folded into the
    projection weights on the host
  - scores/softmax/attn*v are computed with DVE/ACT elementwise ops in
    the rows-on-partitions layout; the mask is pre-broadcast on host
"""

import os

import numpy as np
from contextlib import ExitStack

import concourse.bass as bass
import concourse.mybir as mybir
import concourse.tile as tile
from concourse import bacc
from concourse.bass import ts
from concourse.bass_utils import run_bass_kernel_spmd
from concourse.masks import make_identity

P = 128
N_CAM, G, HEADS, DH, D = 6, 8, 4, 64, 256
NCORES = 8
QLEN = 4096
S = QLEN // NCORES          # 512 positions per core
NST = S // P                # 4 s-tiles per core
NG = N_CAM * G              # 48
FREE_SC = HEADS * NG        # 192
EPS = 1e-5
SCALE = DH ** -0.5
F32 = mybir.dt.float32
F32R = mybir.dt.float32r
AX = mybir.AxisListType
ALU = mybir.AluOpType
ACTF = mybir.ActivationFunctionType

_PROGRAM_CACHE = {}


def _build_program():
    nc = bacc.Bacc(
        "TRN2",
        target_bir_lowering=False,
        debug=False,
        enable_asserts=False,
        num_devices=NCORES,
    )

    qx_d = nc.dram_tensor("qx", (N_CAM * S, D), F32, kind="ExternalInput")
    kx_d = nc.dram_tensor("kx", (NG * S, D), F32, kind="ExternalInput")
    vx_d = nc.dram_tensor("vx", (NG * S, D), F32, kind="ExternalInput")
    am_d = nc.dram_tensor("amask", (S, FREE_SC), F32, kind="ExternalInput")
    sk_d = nc.dram_tensor("skipx", (S, D), F32, kind="ExternalInput")
    wq_d = nc.dram_tensor("wq", (2, P, D), F32, kind="ExternalInput")
    wk_d = nc.dram_tensor("wk", (2, P, D), F32, kind="ExternalInput")
    wv_d = nc.dram_tensor("wv", (2, P, D), F32, kind="ExternalInput")
    wp_d = nc.dram_tensor("wp", (2, P, D), F32, kind="ExternalInput")
    w1_d = nc.dram_tensor("w1", (2, P, 2 * D), F32, kind="ExternalInput")
    w2_d = nc.dram_tensor("w2", (4, P, D), F32, kind="ExternalInput")
    out_d = nc.dram_tensor("out", (S, D), F32, kind="ExternalOutput")

    with tile.TileContext(nc) as tc, ExitStack() as ctx:
        const = ctx.enter_context(tc.tile_pool(name="const", bufs=1))
        xin_p = ctx.enter_context(tc.tile_pool(name="xin", bufs=10))
        st_p = ctx.enter_context(tc.tile_pool(name="stats", bufs=24))
        xn_p = ctx.enter_context(tc.tile_pool(name="xn", bufs=8))
        xt_p = ctx.enter_context(tc.tile_pool(name="xt", bufs=8))
        pr_p = ctx.enter_context(tc.tile_pool(name="pr", bufs=8))
        pt_p = ctx.enter_context(tc.tile_pool(name="ptr", bufs=3, space="PSUM"))
        pm_p = ctx.enter_context(tc.tile_pool(name="pmm", bufs=3, space="PSUM"))
        pl_p = ctx.enter_context(tc.tile_pool(name="pmlp", bufs=2, space="PSUM"))
        qp_p = ctx.enter_context(tc.tile_pool(name="qp", bufs=N_CAM * NST))
        kp_p = ctx.enter_context(tc.tile_pool(name="kp", bufs=4))
        vp_p = ctx.enter_context(tc.tile_pool(name="vp", bufs=56))
        sc_p = ctx.enter_context(tc.tile_pool(name="sc", bufs=3))
        sm_p = ctx.enter_context(tc.tile_pool(name="sm", bufs=4))
        ac_p = ctx.enter_context(tc.tile_pool(name="acc", bufs=2))
        po_p = ctx.enter_context(tc.tile_pool(name="post", bufs=2))

        ident_f = const.tile([P, P], F32, tag="ident_f")
        make_identity(nc, ident_f[:])
        ident = const.tile([P, P], F32R, tag="ident")
        nc.any.tensor_copy(ident[:], ident_f[:])
        identr = ident[:]
        eps_t = const.tile([P, 1], F32, tag="eps")
        nc.any.memset(eps_t[:], EPS)

        def load_w(d, kt, nn, name):
            stg = const.tile([P, kt, nn], F32, tag="wstg", name=f"stg_{name}")
            nc.sync.dma_start(stg[:], d.ap().rearrange("t p n -> p t n"))
            t = const.tile([P, kt, nn], F32R, tag=name)
            nc.any.tensor_copy(t[:], stg[:])
            return t

        wq_t = load_w(wq_d, 2, D, "wq")
        wk_t = load_w(wk_d, 2, D, "wk")
        wv_t = load_w(wv_d, 2, D, "wv")
        wp_t = load_w(wp_d, 2, D, "wp")
        w1_t = load_w(w1_d, 2, 2 * D, "w1")
        w2_t = load_w(w2_d, 4, D, "w2")

        def ln_stats(x):
            """Returns agg tile; [:,3:4]=rstd, [:,2:3]=-mean*rstd."""
            bns = st_p.tile([P, 6], F32, tag="bns")
            nc.vector.bn_stats(bns[:], x[:])
            agg = st_p.tile([P, 4], F32, tag="agg")
            nc.vector.bn_aggr(agg[:, 0:2], bns[:])
            nc.scalar.activation(agg[:, 2:3], agg[:, 1:2], ACTF.Sqrt, bias=eps_t[:])
            nc.vector.reciprocal(agg[:, 3:4], agg[:, 2:3])
            nc.vector.tensor_scalar(
                agg[:, 2:3], agg[:, 0:1], agg[:, 3:4], -1.0,
                op0=ALU.mult, op1=ALU.mult,
            )
            return agg

        def ln_normalize(x, out_pool, tag):
            agg = ln_stats(x)
            xn = out_pool.tile([P, D], F32R, tag=tag)
            nc.any.tensor_scalar(
                xn[:], x[:], agg[:, 3:4], agg[:, 2:3],
                op0=ALU.mult, op1=ALU.add,
            )
            return xn

        def transpose_to_sbuf(xn, nk):
            """[P, nk*128] rows-major tile -> [P, nk*128] transposed tile."""
            pt = pt_p.tile([P, nk * P], F32, tag="pt")
            for t in range(nk):
                nc.tensor.transpose(
                    pt[:, ts(t, P)].bitcast(F32R),
                    xn[:, ts(t, P)].bitcast(F32R),
                    identr,
                )
            xt = xt_p.tile([P, nk * P], F32R, tag="xt")
            nc.any.tensor_copy(xt[:], pt[:])
            return xt

        def proj_matmul(xt, w_t, nk, nn, psum_pool):
            ps = psum_pool.tile([P, nn], F32, tag=f"ps{nn}")
            for t in range(nk):
                nc.tensor.matmul(
                    ps[:],
                    lhsT=xt[:, ts(t, P)],
                    rhs=w_t[:, t, :],
                    start=(t == 0),
                    stop=(t == nk - 1),
                )
            return ps

        def ln_proj(src_ap, w_t, out_pool, tag):
            """DMA row-tile, LN (no gain/bias: folded in W), project."""
            x = xin_p.tile([P, D], F32, tag="xin")
            nc.sync.dma_start(x[:], src_ap)
            xn = ln_normalize(x, xn_p, "xn")
            xt = transpose_to_sbuf(xn, 2)
            ps = proj_matmul(xt, w_t, 2, D, pm_p)
            out = out_pool.tile([P, D], F32, tag=tag)
            nc.any.tensor_copy(out[:], ps[:])
            return out

        # ---- Phase Q: 24 projected q tiles, resident ----
        qp_tiles = {}
        for n in range(N_CAM):
            for st in range(NST):
                row0 = n * S + st * P
                qp_tiles[(n, st)] = ln_proj(
                    qx_d.ap()[row0:row0 + P, :], wq_t, qp_p, "qp"
                )

        # ---- Main: per s-tile ----
        for st in range(NST):
            sc = sc_p.tile([P, HEADS, N_CAM, G], F32, tag="sc")
            vp_tiles = {}
            for n in range(N_CAM):
                qpt = qp_tiles[(n, st)]
                for g in range(G):
                    blk = (n * G + g) * S + st * P
                    kp = ln_proj(kx_d.ap()[blk:blk + P, :], wk_t, kp_p, "kp")
                    vp = ln_proj(vx_d.ap()[blk:blk + P, :], wv_t, vp_p, "vp")
                    vp_tiles[(n, g)] = vp
                    prod = pr_p.tile([P, D], F32, tag="prod")
                    nc.gpsimd.tensor_tensor(prod[:], kp[:], qpt[:], op=ALU.mult)
                    nc.vector.tensor_reduce(
                        sc[:, :, n, g],
                        prod[:].rearrange("p (m d) -> p m d", m=HEADS),
                        op=ALU.add,
                        axis=AX.X,
                    )

            # mask + softmax over (n, g) per head
            am = xin_p.tile([P, HEADS, N_CAM, G], F32, tag="am")
            nc.sync.dma_start(am[:], am_d.ap()[ts(st, P), :])
            nc.gpsimd.tensor_tensor(sc[:], sc[:], am[:], op=ALU.add)
            nm = sm_p.tile([P, HEADS], F32, tag="nm")
            nc.vector.tensor_reduce(
                nm[:],
                sc[:].rearrange("p m n g -> p m (n g)"),
                op=ALU.max,
                axis=AX.X,
                negate=True,
            )
            att = sc_p.tile([P, HEADS, N_CAM, G], F32, tag="att")
            se = sm_p.tile([P, HEADS], F32, tag="se")
            for m in range(HEADS):
                nc.scalar.activation(
                    att[:, m], sc[:, m], ACTF.Exp,
                    bias=nm[:, m:m + 1], accum_out=se[:, m:m + 1],
                )
            rc = sm_p.tile([P, HEADS], F32, tag="rc")
            nc.vector.reciprocal(rc[:], se[:])
            for m in range(HEADS):
                nc.any.tensor_scalar_mul(att[:, m], att[:, m], rc[:, m:m + 1])

            # attn @ v : 4 parallel accumulation chains
            accs = [
                ac_p.tile([P, D], F32, tag=f"acc{j}", name=f"acc{j}_{st}")
                for j in range(4)
            ]
            idx = 0
            for n in range(N_CAM):
                for g in range(G):
                    vp = vp_tiles.pop((n, g))
                    j, r = divmod(idx, 12)
                    attb = att[:, :, n, g][:, :, None].broadcast_to(
                        (P, HEADS, DH)
                    )
                    vpv = vp[:].rearrange("p (m d) -> p m d", m=HEADS)
                    accv = accs[j][:].rearrange("p (m d) -> p m d", m=HEADS)
                    if r == 0:
                        nc.any.tensor_tensor(accv, attb, vpv, op=ALU.mult)
                    else:
                        prod2 = pr_p.tile([P, D], F32, tag="prod2")
                        p2v = prod2[:].rearrange("p (m d) -> p m d", m=HEADS)
                        nc.any.tensor_tensor(p2v, attb, vpv, op=ALU.mult)
                        eng = nc.gpsimd if (j % 2 == 0) else nc.any
                        eng.tensor_tensor(
                            accs[j][:], accs[j][:], prod2[:], op=ALU.add
                        )
                    idx += 1
            nc.any.tensor_tensor(accs[0][:], accs[0][:], accs[1][:], op=ALU.add)
            nc.any.tensor_tensor(accs[2][:], accs[2][:], accs[3][:], op=ALU.add)
            a_t = ac_p.tile([P, D], F32R, tag="a")
            nc.any.tensor_tensor(a_t[:], accs[0][:], accs[2][:], op=ALU.add)

            # ---- post-attention: proj + skip, ln_pre, mlp, ln_post ----
            at = transpose_to_sbuf(a_t, 2)
            ps = proj_matmul(at, wp_t, 2, D, pm_p)
            sk = xin_p.tile([P, D], F32, tag="sk")
            nc.sync.dma_start(sk[:], sk_d.ap()[ts(st, P), :])
            z = po_p.tile([P, D], F32, tag="z")
            nc.any.tensor_tensor(z[:], ps[:], sk[:], op=ALU.add)

            zn = ln_normalize(z, po_p, "zn")

            znt = transpose_to_sbuf(zn, 2)
            ps1 = proj_matmul(znt, w1_t, 2, 2 * D, pl_p)
            h1 = po_p.tile([P, 2 * D], F32R, tag="h1")
            nc.scalar.activation(h1[:], ps1[:], ACTF.Gelu)

            h1t = transpose_to_sbuf(h1, 4)
            ps2 = proj_matmul(h1t, w2_t, 4, D, pm_p)
            z2 = po_p.tile([P, D], F32, tag="z2")
            nc.any.tensor_tensor(z2[:], ps2[:], zn[:].bitcast(F32), op=ALU.add)

            zo = ln_normalize(z2, po_p, "zo")
            nc.sync.dma_start(out_d.ap()[ts(st, P), :], zo[:].bitcast(F32))

    if not os.environ.get("KERNEL_SKIP_COMPILE"):
        nc.compile()
    return nc


def _get_program():
    if "p" not in _PROGRAM_CACHE:
        _PROGRAM_CACHE["p"] = _build_program()
    return _PROGRAM_CACHE["p"]


def kernel(q, k, v, skip, mask,
           ln_q_g, ln_q_b, wq, bq,
           ln_k_g, ln_k_b, wk, bk,
           ln_v_g, ln_v_b, wv, bv,
           w_proj, b_proj,
           ln_pre_g, ln_pre_b,
           w_mlp1, b_mlp1, w_mlp2, b_mlp2,
           ln_post_g, ln_post_b):
    q = np.asarray(q, np.float32)
    k = np.asarray(k, np.float32)
    v = np.asarray(v, np.float32)
    skip = np.asarray(skip, np.float32)
    mask = np.asarray(mask)

    # fold LN gains (and attention scale for q) into projection weights;
    # the corresponding biases are all zero in this model instance --
    # assert rather than silently drop them.
    f = np.float32
    wqf = (np.asarray(ln_q_g)[:, None] * np.asarray(wq) * SCALE).astype(f)
    wkf = (np.asarray(ln_k_g)[:, None] * np.asarray(wk)).astype(f)
    wvf = (np.asarray(ln_v_g)[:, None] * np.asarray(wv)).astype(f)
    for name, val in [
        ("bq'", np.asarray(ln_q_b) @ np.asarray(wq) + np.asarray(bq)),
        ("bk'", np.asarray(ln_k_b) @ np.asarray(wk) + np.asarray(bk)),
        ("bv'", np.asarray(ln_v_b) @ np.asarray(wv) + np.asarray(bv)),
        ("b_proj", np.asarray(b_proj)),
        ("b_mlp1", np.asarray(b_mlp1)),
        ("b_mlp2", np.asarray(b_mlp2)),
        ("ln_pre_b", np.asarray(ln_pre_b)),
        ("ln_post_b", np.asarray(ln_post_b)),
    ]:
        assert np.allclose(val, 0.0, atol=1e-12), f"{name} nonzero: unsupported"
    for name, val in [("ln_pre_g", ln_pre_g), ("ln_post_g", ln_post_g)]:
        assert np.allclose(np.asarray(val), 1.0), f"{name} != 1: unsupported"

    wpf = np.ascontiguousarray(np.asarray(w_proj, f))
    w1f = np.ascontiguousarray(np.asarray(w_mlp1, f))
    w2f = np.ascontiguousarray(np.asarray(w_mlp2, f))

    wq_p = np.ascontiguousarray(wqf.reshape(2, P, D))
    wk_p = np.ascontiguousarray(wkf.reshape(2, P, D))
    wv_p = np.ascontiguousarray(wvf.reshape(2, P, D))
    wp_p = np.ascontiguousarray(wpf.reshape(2, P, D))
    w1_p = np.ascontiguousarray(w1f.reshape(2, P, 2 * D))
    w2_p = np.ascontiguousarray(w2f.reshape(4, P, D))

    # host-side data layout prep
    qx_all = np.ascontiguousarray(
        q[0].transpose(0, 2, 3, 1).reshape(N_CAM, QLEN, D)
    )
    skip_all = np.ascontiguousarray(
        skip[0].transpose(1, 2, 0).reshape(QLEN, D)
    )
    mask_all = mask[0, :, :, 0].astype(bool)  # (6, 4096)

    in_maps = []
    for c in range(NCORES):
        sl = slice(c * S, (c + 1) * S)
        qx_c = np.ascontiguousarray(qx_all[:, sl, :]).reshape(N_CAM * S, D)
        kx_c = np.ascontiguousarray(
            k[0][:, sl].transpose(0, 2, 1, 3)
        ).reshape(NG * S, D)
        vx_c = np.ascontiguousarray(
            v[0][:, sl].transpose(0, 2, 1, 3)
        ).reshape(NG * S, D)
        mc = mask_all[:, sl]                       # (6, 512)
        amc = np.where(mc.T, f(0.0), f(-1e9)).astype(f)  # (512, 6)
        am_c = np.ascontiguousarray(
            np.broadcast_to(amc[:, None, :, None], (S, HEADS, N_CAM, G))
        ).reshape(S, FREE_SC)
        in_maps.append({
            "qx": qx_c, "kx": kx_c, "vx": vx_c,
            "amask": am_c,
            "skipx": np.ascontiguousarray(skip_all[sl]),
            "wq": wq_p, "wk": wk_p, "wv": wv_p, "wp": wp_p,
            "w1": w1_p, "w2": w2_p,
        })

    global _LAST_IN_MAPS
    _LAST_IN_MAPS = in_maps
    nc = _get_program()
    res = run_bass_kernel_spmd(nc, in_maps, core_ids=list(range(NCORES)))
    z = np.concatenate([res.results[c]["out"] for c in range(NCORES)], axis=0)
    out = z.reshape(64, 64, D).transpose(2, 0, 1)[None]
    return np.ascontiguousarray(out.astype(np.float32))



# revision 12
# speedup vs baseline: 1.6564x; 1.6564x over previous
"""Trainium2 Bass kernel for the KernelAttention module (v2).

Sharding: 4096 query positions split into 8 blocks of 512 (one per core);
softmax mixes only across (camera, group) at fixed position, so no
collectives are needed.

Design (driven by the TimelineSim cost model):
  - All activations/weights in bf16 (DVE 2x modes, half DMA traffic).
  - Inputs are host-pretransposed to feature-major x^T layout so every
    projection is a transpose-free TensorE matmul: out[rows, dout] =
    xT.T @ W with W resident as the moving operand.  No per-tile PE
    transposes, no PSUM->SBUF operand copies: scores/attention read the
    projection results directly from PSUM.
  - LayerNorm is folded algebraically:
      LN(x)@W' = rstd*( x@W' - mean * colsum(W') )
    row means come free from N=1 matmuls against a 1/256 ones-vector
    (lhsT = xT), row E[x^2] from N=1 matmuls with lhsT = (x*x)^T, and
    the corrections are applied on the scores (k), on the attention
    weights / output (v), and at the single qp copy (q).
  - attn@v accumulates on the TensorE: matmul(lhsT=prod, rhs=identity)
    chains into a feat-major PSUM accumulator, which is then directly
    the lhsT for the output projection. The v-mean correction is one
    extra rank-4 matmul on the same accumulator.
"""

import os

import numpy as np
import ml_dtypes
from contextlib import ExitStack

import concourse.bass as bass
import concourse.mybir as mybir
import concourse.tile as tile
from concourse import bacc
from concourse.bass_utils import run_bass_kernel_spmd
from concourse.masks import make_identity

P = 128
N_CAM, G, HEADS, DH, D = 6, 8, 4, 64, 256
NG = N_CAM * G              # 48 key blocks per position
KT = 2                      # feature k-tiles (256 = 2*128)
NCORES = 8
QLEN = 4096
S = QLEN // NCORES          # 512 positions per core
NST = S // P                # 4 s-tiles of 128 rows
R = P                       # rows per block-tile
NGRP = 6                    # groups of 8 blocks
GRP = 8
FREE_SC = HEADS * NG        # 192
EPS = 1e-5
SCALE = DH ** -0.5
NEG = -30000.0

F32 = mybir.dt.float32
F32R = mybir.dt.float32r
BF16 = mybir.dt.bfloat16
AX = mybir.AxisListType
ALU = mybir.AluOpType
ACTF = mybir.ActivationFunctionType
BF = ml_dtypes.bfloat16

_PROGRAM_CACHE = {}


def _build_program():
    nc = bacc.Bacc(
        "TRN2",
        target_bir_lowering=False,
        debug=False,
        enable_asserts=False,
        num_devices=NCORES,
    )

    kT_d = nc.dram_tensor("kT", (KT, P, NST * NG * R), BF16, kind="ExternalInput")
    vT_d = nc.dram_tensor("vT", (KT, P, NST * NG * R), BF16, kind="ExternalInput")
    qT_d = nc.dram_tensor("qT", (KT, P, NST * N_CAM * R), BF16, kind="ExternalInput")
    am_d = nc.dram_tensor("amask", (S, N_CAM), BF16, kind="ExternalInput")
    sk_d = nc.dram_tensor("skipx", (S, D), F32, kind="ExternalInput")
    wq_d = nc.dram_tensor("wq", (KT, P, D), BF16, kind="ExternalInput")
    wk_d = nc.dram_tensor("wk", (KT, P, D), BF16, kind="ExternalInput")
    wv_d = nc.dram_tensor("wv", (KT, P, D), BF16, kind="ExternalInput")
    wp_d = nc.dram_tensor("wp", (KT, P, D), BF16, kind="ExternalInput")
    w1_d = nc.dram_tensor("w1", (KT, P, 2 * D), BF16, kind="ExternalInput")
    w2_d = nc.dram_tensor("w2", (4, P, D), BF16, kind="ExternalInput")
    cqbc_d = nc.dram_tensor("cqbc", (P, D), BF16, kind="ExternalInput")
    ckbc_d = nc.dram_tensor("ckbc", (P, D), BF16, kind="ExternalInput")
    cvsg_d = nc.dram_tensor("cvseg", (HEADS, KT * P), BF16, kind="ExternalInput")
    out_d = nc.dram_tensor("out", (S, D), F32, kind="ExternalOutput")

    with tile.TileContext(nc) as tc, ExitStack() as ctx:
        const = ctx.enter_context(tc.tile_pool(name="const", bufs=1))
        slab_p = ctx.enter_context(tc.tile_pool(name="slab", bufs=2))
        sq_p = ctx.enter_context(tc.tile_pool(name="sq", bufs=4))
        pr_p = ctx.enter_context(tc.tile_pool(name="pr", bufs=4))
        st_p = ctx.enter_context(tc.tile_pool(name="st", bufs=2))
        sm_p = ctx.enter_context(tc.tile_pool(name="sm", bufs=2))
        po_p = ctx.enter_context(tc.tile_pool(name="po", bufs=2))
        pk_ps = ctx.enter_context(tc.tile_pool(name="pk", bufs=2, space="PSUM"))
        pv_ps = ctx.enter_context(tc.tile_pool(name="pv", bufs=2, space="PSUM"))
        ac_ps = ctx.enter_context(tc.tile_pool(name="ac", bufs=1, space="PSUM"))
        ss_ps = ctx.enter_context(tc.tile_pool(name="ss", bufs=1, space="PSUM"))
        tr_ps = ctx.enter_context(tc.tile_pool(name="tr", bufs=1, space="PSUM"))

        # ---------- constants ----------
        identf = const.tile([P, P], F32, tag="identf", name="identf")
        make_identity(nc, identf[:])
        ident = const.tile([P, P], BF16, tag="ident", name="ident")
        nc.vector.tensor_copy(ident[:], identf[:])
        identr_t = const.tile([P, P], F32R, tag="identr", name="identr")
        nc.vector.tensor_copy(identr_t[:], identf[:])
        identr = identr_t[:]
        ones = const.tile([P, 1], BF16, tag="ones", name="ones")
        nc.any.memset(ones[:], 1.0 / D)
        eps_t = const.tile([P, 1], F32, tag="eps", name="eps")
        nc.any.memset(eps_t[:], EPS)

        def load_const(d, shape, name, rearr=None, **kw):
            t = const.tile(shape, BF16, tag=name, name=name)
            ap = d.ap()
            if rearr:
                ap = ap.rearrange(rearr, **kw)
            nc.sync.dma_start(t[:], ap)
            return t

        wq_t = load_const(wq_d, [P, KT, D], "wq", "t p n -> p t n")
        wk_t = load_const(wk_d, [P, KT, D], "wk", "t p n -> p t n")
        wv_t = load_const(wv_d, [P, KT, D], "wv", "t p n -> p t n")
        wp_t = load_const(wp_d, [P, KT, D], "wp", "t p n -> p t n")
        w1_t = load_const(w1_d, [P, KT, 2 * D], "w1", "t p n -> p t n")
        w2_t = load_const(w2_d, [P, 4, D], "w2", "t p n -> p t n")
        cqbc = load_const(cqbc_d, [P, D], "cqbc")
        ckbc = load_const(ckbc_d, [P, D], "ckbc")
        cvsg = load_const(cvsg_d, [HEADS, KT * P], "cvseg")
        amask = load_const(am_d, [P, NST, N_CAM], "amask", "(s p) n -> p s n", s=NST)
        skip_t = const.tile([P, NST, D], F32, tag="skip", name="skip")
        nc.sync.dma_start(skip_t[:], sk_d.ap().rearrange("(s p) d -> p s d", s=NST))
        qslab = const.tile([P, KT, NST * N_CAM * R], BF16, tag="qslab", name="qslab")
        nc.sync.dma_start(qslab[:], qT_d.ap().rearrange("t p x -> p t x"))

        zo_all = const.tile([P, NST, D], F32, tag="zo", name="zo")

        # ---------- helpers ----------
        def stat_mms(xv, sqv, psum, col):
            """mean -> psum[:, col], E[x^2] -> psum[:, col+1] (N=1 matmuls).

            xv: [P, KT, R] view; sqv: [P, KT*R] tile (bf16)."""
            for t in range(KT):
                nc.tensor.matmul(psum[:, col:col + 1], lhsT=xv[:, t],
                                 rhs=ones[:], start=(t == 0), stop=(t == KT - 1))
            for t in range(KT):
                nc.tensor.matmul(psum[:, col + 1:col + 2],
                                 lhsT=sqv[:, t * R:(t + 1) * R],
                                 rhs=ones[:], start=(t == 0), stop=(t == KT - 1))

        def stats_finish(stats, n, rstd_out, eng=None):
            """stats: [P, 2n] sbuf f32 (mean, ex2 interleaved). Returns
            (mean_view, rstd_out). var computed in-place into ex2 slots."""
            sv = stats[:].rearrange("p (b two) -> p b two", two=2)
            mean_v = sv[:, :, 0]
            ex2_v = sv[:, :, 1]
            tmp = sm_p.tile([P, n], F32, tag="vtmp", name=f"vtmp{n}")
            e = eng or nc.vector
            e.tensor_tensor(tmp[:], mean_v, mean_v, op=ALU.mult)
            e.tensor_tensor(ex2_v, ex2_v, tmp[:], op=ALU.subtract)
            nc.scalar.activation(tmp[:], ex2_v, ACTF.Sqrt, bias=eps_t[:])
            nc.vector.reciprocal(rstd_out[:], tmp[:])
            return mean_v

        # ---------- Q phase ----------
        qp_tiles = {}
        qstat = ss_ps.tile([P, 48], F32, tag="sstat", name="qstat")
        sqq_tiles = {}
        for st in range(NST):
            for cam in range(N_CAM):
                i = st * N_CAM + cam
                xq = qslab[:, :, i * R:(i + 1) * R]
                sqq = sq_p.tile([P, KT * R], BF16, tag="sq", name=f"sqq{i}")
                nc.vector.tensor_tensor(
                    sqq[:].rearrange("p (t r) -> p t r", t=KT), xq, xq,
                    op=ALU.mult)
                stat_mms(xq, sqq, qstat, 2 * i)
        qstats = st_p.tile([P, 2 * NST * N_CAM], F32, tag="qstats", name="qstats")
        nc.vector.tensor_copy(qstats[:], qstat[:])
        rstd_q = st_p.tile([P, NST * N_CAM], F32, tag="rstdq", name="rstdq")
        mq_v = stats_finish(qstats, NST * N_CAM, rstd_q)

        t_all = const.tile([P, NST, HEADS, N_CAM], F32, tag="tall", name="tall")
        for st in range(NST):
            for cam in range(N_CAM):
                i = st * N_CAM + cam
                xq = qslab[:, :, i * R:(i + 1) * R]
                yq = pk_ps.tile([P, 2 * D], F32, tag="pk", name=f"yq{i}")
                for t in range(KT):
                    nc.tensor.matmul(yq[:, 0:D], lhsT=xq[:, t], rhs=wq_t[:, t],
                                     start=(t == 0), stop=(t == KT - 1))
                # qp = rstd_q * Yq - (rstd_q*mq) * cq
                qp = const.tile([P, D], BF16, tag=f"qp{i}", name=f"qp{i}")
                nc.scalar.activation(qp[:], yq[:, 0:D], ACTF.Copy,
                                     scale=rstd_q[:, i:i + 1])
                mr = sm_p.tile([P, 1], F32, tag="mrq", name=f"mrq{i}")
                nc.vector.tensor_tensor(mr[:], mq_v[:, i:i + 1],
                                        rstd_q[:, i:i + 1], op=ALU.mult)
                tmp = pr_p.tile([P, D], BF16, tag="prodq", name=f"qc{i}")
                nc.vector.tensor_scalar(tmp[:], cqbc[:], mr[:], None,
                                        op0=ALU.mult)
                nc.vector.tensor_tensor(qp[:], qp[:], tmp[:], op=ALU.subtract)
                qp_tiles[(st, cam)] = qp
                # t-tile: per-head sum(qp * ck)
                prd = pr_p.tile([P, D], BF16, tag="prodq", name=f"qt{i}")
                nc.gpsimd.tensor_tensor(prd[:], qp[:], ckbc[:], op=ALU.mult)
                nc.vector.tensor_reduce(
                    t_all[:, st, :, cam],
                    prd[:].rearrange("p (m d) -> p m d", m=HEADS),
                    op=ALU.add, axis=AX.X)

        # ---------- main s-tile loop ----------
        for st in range(NST):
            kslab = slab_p.tile([P, KT, NG * R], BF16, tag="kslab",
                                name=f"kslab{st}")
            nc.sync.dma_start(
                kslab[:],
                kT_d.ap()[:, :, st * NG * R:(st + 1) * NG * R]
                .rearrange("t p x -> p t x"))
            vslab = slab_p.tile([P, KT, NG * R], BF16, tag="vslab",
                                name=f"vslab{st}")
            nc.sync.dma_start(
                vslab[:],
                vT_d.ap()[:, :, st * NG * R:(st + 1) * NG * R]
                .rearrange("t p x -> p t x"))

            kstats = st_p.tile([P, 2 * NG], F32, tag="kstats", name=f"kst{st}")
            vstats = st_p.tile([P, 2 * NG], F32, tag="vstats", name=f"vst{st}")
            sc = sm_p.tile([P, HEADS, NG], F32, tag="sc", name=f"sc{st}")

            # ---- K pass: stats (k and v) + k-proj + raw scores ----
            opidx = 0
            for grp in range(NGRP):
                stat_g = ss_ps.tile([P, 48], F32, tag="sstat",
                                    name=f"stat{st}_{grp}")
                pk_pair = {}
                for j in range(GRP):
                    blk = grp * GRP + j
                    xk = kslab[:, :, blk * R:(blk + 1) * R]
                    xv = vslab[:, :, blk * R:(blk + 1) * R]
                    # squares (engine alternation)
                    sqk = sq_p.tile([P, KT * R], BF16, tag="sq",
                                    name=f"sqk{st}_{blk}")
                    sqv = sq_p.tile([P, KT * R], BF16, tag="sq",
                                    name=f"sqv{st}_{blk}")
                    sqk_v = sqk[:].rearrange("p (t r) -> p t r", t=KT)
                    sqv_v = sqv[:].rearrange("p (t r) -> p t r", t=KT)
                    if opidx % 3 == 2:
                        nc.scalar.activation(sqk_v, xk, ACTF.Square)
                        nc.scalar.activation(sqv_v, xv, ACTF.Square)
                    else:
                        nc.vector.tensor_tensor(sqk_v, xk, xk, op=ALU.mult)
                        nc.vector.tensor_tensor(sqv_v, xv, xv, op=ALU.mult)
                    opidx += 1
                    stat_mms(xk, sqk, stat_g, 2 * j)
                    stat_mms(xv, sqv, stat_g, 16 + 2 * j)
                    # k projection (pairs share a psum bank tile)
                    pair = j // 2
                    if j % 2 == 0:
                        pk_pair[pair] = pk_ps.tile([P, 2 * D], F32, tag="pk",
                                                   name=f"pk{st}_{grp}_{pair}")
                    yk = pk_pair[pair]
                    off = (j % 2) * D
                    for t in range(KT):
                        nc.tensor.matmul(yk[:, off:off + D], lhsT=xk[:, t],
                                         rhs=wk_t[:, t],
                                         start=(t == 0), stop=(t == KT - 1))
                    if j % 2 == 1:
                        cam = blk // G
                        ng0 = grp * GRP + pair * 2
                        qp = qp_tiles[(st, cam)]
                        prodk = pr_p.tile([P, 2, D], BF16, tag="prod",
                                          name=f"prk{st}_{grp}_{pair}")
                        qp_bc = qp[:][:, None, :].broadcast_to((P, 2, D))
                        nc.vector.tensor_tensor(
                            prodk[:], qp_bc,
                            yk[:].rearrange("p (b f) -> p b f", b=2),
                            op=ALU.mult)
                        # reduce over dh per head -> sc[:, m, ng0:ng0+2]
                        nc.vector.tensor_reduce(
                            sc[:, :, ng0:ng0 + 2].rearrange("p m b -> p b m"),
                            prodk[:].rearrange("p b (m d) -> p b m d", m=HEADS),
                            op=ALU.add, axis=AX.X)
                nc.vector.tensor_copy(kstats[:, grp * 16:(grp + 1) * 16],
                                      stat_g[:, 0:16])
                nc.vector.tensor_copy(vstats[:, grp * 16:(grp + 1) * 16],
                                      stat_g[:, 16:32])

            # ---- stats math ----
            rstd_k = st_p.tile([P, NG], F32, tag="rstdk", name=f"rsk{st}")
            mk_v = stats_finish(kstats, NG, rstd_k)
            rstd_v = st_p.tile([P, NG], F32, tag="rstdv", name=f"rsv{st}")
            mv_v = stats_finish(vstats, NG, rstd_v, eng=nc.gpsimd)

            # ---- softmax over (cam, g) per head ----
            # corrected scores: sc4 = rstd_k*(sc - mk*t) + mask
            mkt = sm_p.tile([P, HEADS, N_CAM, G], F32, tag="mkt", name=f"mkt{st}")
            t_bc = t_all[:, st][:, :, :, None].broadcast_to((P, HEADS, N_CAM, G))
            mk_bc = mk_v[:, None, :].rearrange(
                "p m (n g) -> p m n g", n=N_CAM).broadcast_to((P, HEADS, N_CAM, G))
            nc.gpsimd.tensor_tensor(mkt[:], t_bc, mk_bc, op=ALU.mult)
            scv = sc[:].rearrange("p m (n g) -> p m n g", n=N_CAM)
            nc.vector.tensor_tensor(scv, scv, mkt[:], op=ALU.subtract)
            rk_bc = rstd_k[:, None, :].rearrange(
                "p m (n g) -> p m n g", n=N_CAM).broadcast_to((P, HEADS, N_CAM, G))
            nc.gpsimd.tensor_tensor(scv, scv, rk_bc, op=ALU.mult)
            am_bc = amask[:, st][:, None, :, None].broadcast_to(
                (P, HEADS, N_CAM, G))
            nc.vector.tensor_tensor(scv, scv, am_bc, op=ALU.add)

            nm = sm_p.tile([P, HEADS], F32, tag="nm", name=f"nm{st}")
            nc.vector.tensor_reduce(nm[:], sc[:], op=ALU.max, axis=AX.X,
                                    negate=True)
            att = sm_p.tile([P, HEADS, NG], BF16, tag="att", name=f"att{st}")
            se = sm_p.tile([P, HEADS], F32, tag="se", name=f"se{st}")
            for m in range(HEADS):
                nc.scalar.activation(att[:, m], sc[:, m], ACTF.Exp,
                                     bias=nm[:, m:m + 1],
                                     accum_out=se[:, m:m + 1])
            rc = sm_p.tile([P, HEADS], F32, tag="rc", name=f"rc{st}")
            nc.vector.reciprocal(rc[:], se[:])
            # attw = att * (rc[m] * rstd_v[ng]); attw2 = attw * mv
            wmul = sm_p.tile([P, HEADS, NG], F32, tag="wmul", name=f"wm{st}")
            for m in range(HEADS):
                nc.vector.tensor_scalar(wmul[:, m], rstd_v[:], rc[:, m:m + 1],
                                        None, op0=ALU.mult)
            attw = sm_p.tile([P, HEADS, NG], BF16, tag="attw", name=f"aw{st}")
            nc.vector.tensor_tensor(attw[:], att[:], wmul[:], op=ALU.mult)
            attw2 = sm_p.tile([P, HEADS, NG], F32, tag="attw2", name=f"aw2{st}")
            mv_bc = mv_v[:, None, :].broadcast_to((P, HEADS, NG))
            nc.gpsimd.tensor_tensor(attw2[:], attw[:], mv_bc, op=ALU.mult)
            w2s = sm_p.tile([P, HEADS], F32R, tag="w2s", name=f"w2s{st}")
            with nc.allow_low_precision(reason="f32r is 32-bit"):
                nc.vector.tensor_reduce(w2s[:], attw2[:], op=ALU.add,
                                        axis=AX.X)
            w2sT_ps = tr_ps.tile([P, 2 * D], F32, tag="trb", name=f"w2sT{st}")
            nc.tensor.transpose(w2sT_ps[0:HEADS, 0:P].bitcast(F32R),
                                w2s[:], identr)
            w2sT = sm_p.tile([HEADS, P], BF16, tag="w2sTs", name=f"w2sTs{st}")
            nc.scalar.activation(w2sT[:], w2sT_ps[0:HEADS, 0:P], ACTF.Copy)

            # ---- AV pass: v-proj + weighted accumulation on TensorE ----
            accs = [ac_ps.tile([P, R], F32, tag=f"acc{t}", name=f"acc{t}_{st}")
                    for t in range(KT)]
            pv_pair = {}
            for grp in range(NGRP):
                for j in range(GRP):
                    blk = grp * GRP + j
                    xv = vslab[:, :, blk * R:(blk + 1) * R]
                    pair = j // 2
                    if j % 2 == 0:
                        pv_pair[pair] = pv_ps.tile([P, 2 * D], F32, tag="pv",
                                                   name=f"pv{st}_{grp}_{pair}")
                    yv = pv_pair[pair]
                    off = (j % 2) * D
                    for t in range(KT):
                        nc.tensor.matmul(yv[:, off:off + D], lhsT=xv[:, t],
                                         rhs=wv_t[:, t],
                                         start=(t == 0), stop=(t == KT - 1))
                    if j % 2 == 1:
                        ng0 = grp * GRP + pair * 2
                        prodv = pr_p.tile([P, 2, D], BF16, tag="prod",
                                          name=f"prv{st}_{grp}_{pair}")
                        aw_bc = attw[:, :, ng0:ng0 + 2].rearrange(
                            "p m b -> p b m")[:, :, :, None].broadcast_to(
                            (P, 2, HEADS, DH))
                        nc.vector.tensor_tensor(
                            prodv[:].rearrange("p b (m d) -> p b m d", m=HEADS),
                            aw_bc,
                            yv[:].rearrange("p (b m d) -> p b m d",
                                            b=2, m=HEADS),
                            op=ALU.mult)
                        for jj in range(2):
                            for t in range(KT):
                                nc.tensor.matmul(
                                    accs[t][:],
                                    lhsT=prodv[:, jj, t * P:(t + 1) * P],
                                    rhs=ident[:],
                                    start=(ng0 + jj == 0), stop=False)
            # v-mean correction: acc_t -= cv_seg[t] (x) w2s
            for t in range(KT):
                nc.tensor.matmul(accs[t][:], lhsT=cvsg[:, t * P:(t + 1) * P],
                                 rhs=w2sT[:], start=False, stop=True)

            # ---- post: out proj + skip + LN + MLP + LN ----
            avT = po_p.tile([P, KT, R], BF16, tag="avT", name=f"avT{st}")
            for t in range(KT):
                nc.scalar.activation(avT[:, t], accs[t][:], ACTF.Copy)
            zps = pk_ps.tile([P, 2 * D], F32, tag="pk", name=f"zps{st}")
            for t in range(KT):
                nc.tensor.matmul(zps[:, 0:D], lhsT=avT[:, t], rhs=wp_t[:, t],
                                 start=(t == 0), stop=(t == KT - 1))
            z = po_p.tile([P, D], F32, tag="z", name=f"z{st}")
            nc.vector.tensor_tensor(z[:], zps[:, 0:D], skip_t[:, st],
                                    op=ALU.add)

            def layer_norm(x, out, outdt, tag):
                bns = sm_p.tile([P, 6], F32, tag="bns", name=f"bns{tag}")
                nc.vector.bn_stats(bns[:], x[:])
                agg = sm_p.tile([P, 4], F32, tag="agg", name=f"agg{tag}")
                nc.vector.bn_aggr(agg[:, 0:2], bns[:])
                nc.scalar.activation(agg[:, 2:3], agg[:, 1:2], ACTF.Sqrt,
                                     bias=eps_t[:])
                nc.vector.reciprocal(agg[:, 3:4], agg[:, 2:3])
                nc.vector.tensor_scalar(agg[:, 2:3], agg[:, 0:1],
                                        agg[:, 3:4], -1.0,
                                        op0=ALU.mult, op1=ALU.mult)
                nc.vector.tensor_scalar(out[:], x[:], agg[:, 3:4],
                                        agg[:, 2:3], op0=ALU.mult,
                                        op1=ALU.add)

            zn = po_p.tile([P, D], F32R, tag="zn", name=f"zn{st}")
            layer_norm(z, zn, BF16, f"pre{st}")
            znT_psb = tr_ps.tile([P, 2 * D], F32, tag="trb", name=f"znT{st}")
            znT_ps = znT_psb[:, 0:D]
            for t in range(KT):
                nc.tensor.transpose(znT_ps[:, t * P:(t + 1) * P].bitcast(F32R),
                                    zn[:, t * P:(t + 1) * P],
                                    identr)
            znT = po_p.tile([P, KT, P], BF16, tag="znTs", name=f"znTs{st}")
            nc.scalar.activation(znT[:].rearrange("p t r -> p (t r)"),
                                 znT_ps, ACTF.Copy)
            ps1 = pv_ps.tile([P, 2 * D], F32, tag="pv", name=f"ps1{st}")
            for t in range(KT):
                nc.tensor.matmul(ps1[:], lhsT=znT[:, t], rhs=w1_t[:, t],
                                 start=(t == 0), stop=(t == KT - 1))
            h1 = po_p.tile([P, 2 * D], F32R, tag="h1", name=f"h1{st}")
            nc.scalar.activation(h1[:], ps1[:], ACTF.Gelu)
            h1T_ps = tr_ps.tile([P, 2 * D], F32, tag="trb", name=f"h1T{st}")
            for t in range(4):
                nc.tensor.transpose(h1T_ps[:, t * P:(t + 1) * P].bitcast(F32R),
                                    h1[:, t * P:(t + 1) * P],
                                    identr)
            h1T = po_p.tile([P, 4, P], BF16, tag="h1Ts", name=f"h1Ts{st}")
            nc.vector.tensor_copy(h1T[:].rearrange("p t r -> p (t r)"),
                                  h1T_ps[:])
            ps2 = pk_ps.tile([P, 2 * D], F32, tag="pk", name=f"ps2{st}")
            for t in range(4):
                nc.tensor.matmul(ps2[:, 0:D], lhsT=h1T[:, t], rhs=w2_t[:, t],
                                 start=(t == 0), stop=(t == 3))
            z2 = po_p.tile([P, D], F32, tag="z2", name=f"z2{st}")
            nc.vector.tensor_tensor(z2[:], ps2[:, 0:D], zn[:].bitcast(F32),
                                    op=ALU.add)
            layer_norm(z2, zo_all[:, st], F32, f"post{st}")

        nc.sync.dma_start(out_d.ap().rearrange("(s p) d -> p s d", s=NST),
                          zo_all[:])

    if not os.environ.get("KERNEL_SKIP_COMPILE"):
        nc.compile()
    return nc


def _get_program():
    if "p" not in _PROGRAM_CACHE:
        _PROGRAM_CACHE["p"] = _build_program()
    return _PROGRAM_CACHE["p"]


def kernel(q, k, v, skip, mask,
           ln_q_g, ln_q_b, wq, bq,
           ln_k_g, ln_k_b, wk, bk,
           ln_v_g, ln_v_b, wv, bv,
           w_proj, b_proj,
           ln_pre_g, ln_pre_b,
           w_mlp1, b_mlp1, w_mlp2, b_mlp2,
           ln_post_g, ln_post_b):
    f = np.float32
    q = np.asarray(q, f)
    k = np.asarray(k, f)
    v = np.asarray(v, f)
    skip = np.asarray(skip, f)
    mask = np.asarray(mask)

    # fold LN gains (and attention scale for q) into projection weights;
    # biases are all zero in this model instance -- assert, don't drop.
    wqf = (np.asarray(ln_q_g)[:, None] * np.asarray(wq) * SCALE).astype(f)
    wkf = (np.asarray(ln_k_g)[:, None] * np.asarray(wk)).astype(f)
    wvf = (np.asarray(ln_v_g)[:, None] * np.asarray(wv)).astype(f)
    for name, val in [
        ("bq'", np.asarray(ln_q_b) @ np.asarray(wq) + np.asarray(bq)),
        ("bk'", np.asarray(ln_k_b) @ np.asarray(wk) + np.asarray(bk)),
        ("bv'", np.asarray(ln_v_b) @ np.asarray(wv) + np.asarray(bv)),
        ("b_proj", np.asarray(b_proj)),
        ("b_mlp1", np.asarray(b_mlp1)),
        ("b_mlp2", np.asarray(b_mlp2)),
        ("ln_pre_b", np.asarray(ln_pre_b)),
        ("ln_post_b", np.asarray(ln_post_b)),
    ]:
        assert np.allclose(val, 0.0, atol=1e-12), f"{name} nonzero: unsupported"
    for name, val in [("ln_pre_g", ln_pre_g), ("ln_post_g", ln_post_g)]:
        assert np.allclose(np.asarray(val), 1.0), f"{name} != 1: unsupported"

    cq = wqf.sum(0)
    ck = wkf.sum(0)
    cv = wvf.sum(0)
    # negated, head-masked cv segments: [HEADS, KT*P]
    cvseg = np.zeros((HEADS, KT * P), f)
    for ff in range(D):
        cvseg[ff // DH, ff] = -cv[ff]

    def bf(x):
        return np.ascontiguousarray(x.astype(BF))

    wq_p = bf(wqf.reshape(KT, P, D))
    wk_p = bf(wkf.reshape(KT, P, D))
    wv_p = bf(wvf.reshape(KT, P, D))
    wp_p = bf(np.asarray(w_proj, f).reshape(KT, P, D))
    w1_p = bf(np.asarray(w_mlp1, f).reshape(KT, P, 2 * D))
    w2_p = bf(np.asarray(w_mlp2, f).reshape(4, P, D))
    cqbc = bf(np.broadcast_to(cq, (P, D)))
    ckbc = bf(np.broadcast_to(ck, (P, D)))
    cvsg = bf(cvseg)

    # host-side data layout prep
    qx = q[0].transpose(0, 2, 3, 1).reshape(N_CAM, QLEN, D)   # (cam, pos, d)
    skip_all = skip[0].transpose(1, 2, 0).reshape(QLEN, D)
    mask_all = mask[0, :, :, 0].astype(bool)                  # (cam, pos)

    in_maps = []
    for c in range(NCORES):
        sl = slice(c * S, (c + 1) * S)
        # qT: [KT, P, (st, cam, r)]
        qc = qx[:, sl].reshape(N_CAM, NST, R, D)
        qT = bf(qc.transpose(3, 1, 0, 2).reshape(KT, P, NST * N_CAM * R))
        # kT/vT: [KT, P, (st, cam, g, r)]
        kc = k[0][:, sl].reshape(N_CAM, NST, R, G, D)
        kT = bf(kc.transpose(4, 1, 0, 3, 2).reshape(KT, P, NST * NG * R))
        vc = v[0][:, sl].reshape(N_CAM, NST, R, G, D)
        vT = bf(vc.transpose(4, 1, 0, 3, 2).reshape(KT, P, NST * NG * R))
        amc = np.where(mask_all[:, sl].T, f(0.0), f(NEG)).astype(BF)  # (S, cam)
        in_maps.append({
            "kT": kT, "vT": vT, "qT": qT,
            "amask": np.ascontiguousarray(amc),
            "skipx": np.ascontiguousarray(skip_all[sl]),
            "wq": wq_p, "wk": wk_p, "wv": wv_p, "wp": wp_p,
            "w1": w1_p, "w2": w2_p,
            "cqbc": cqbc, "ckbc": ckbc, "cvseg": cvsg,
        })

    global _LAST_IN_MAPS
    _LAST_IN_MAPS = in_maps
    nc = _get_program()
    res = run_bass_kernel_spmd(nc, in_maps, core_ids=list(range(NCORES)))
    z = np.concatenate([res.results[c]["out"] for c in range(NCORES)], axis=0)
    out = z.reshape(64, 64, D).transpose(2, 0, 1)[None]
    return np.ascontiguousarray(out.astype(np.float32))
